# revision 77
# baseline (speedup 1.0000x reference)
"""Trainium2 Bass kernel for nn_Attention_32762010534254.

Cross-attention: q(B,Nq,D) kv(B,Nkv,D) -> softmax((qWq)(kvWk)^T/sqrt(dh)) (kvWv) Wo + bo
B=2, Nq=512, Nkv=4096, D=1024, heads=16, dh=64.

Sharding (8 cores): core i handles batch b=i//4 and head group g=i%4
(4 heads = 2 pairs). Per-core device program (~116.9us in the cost model
vs a ~103.4us PE-busy floor at 1 cycle/row):
  - inputs DMA'd as bf16 (host-side cast); every matmul streams 1
    cycle/row (f32r needs moving-N>=256, bf16 always)
  - K^T / V projections streamed over Nkv in 512-key chunks; projection
    pieces run ~6 pieces ahead of the attention pieces that consume them
    so the PE never waits on a projection; chunk-0 projections spread
    their PSUM across the not-yet-used QK banks to avoid WAR stalls
  - S^T = K_h Q_h^T with keys on partitions (row-tiled head pairs, K=64),
    Q^T/K^T kept f32r for precision at no PE cost
  - exp via ACT with fused 1/8 scale + per-key mask bias -> P^T bf16
  - AV in O[q,dh] orientation: stationary P^T q-slices, moving V_aug
    (dh + ones col -> softmax sums ride along) bf16, N=65: half the
    moving rows of the O^T formulation. 16 accumulators (4 heads x 4
    q-tiles) packed into 4 pair-pure PSUM banks (7+1 slots per head
    pair) via pending-zero semantics (the first matmul into each bank
    carries start=True).
  - AV entries are hoarded mid-stream (head pair 1 drains first) and
    spent as PE filler over the last, projection-free, ACT-bound pieces;
    chunk 7 runs pair-major so pair-1's normalize/transpose overlaps
    pair-0's QK/exp window
  - tail: strided per-bank reciprocals of the sums, X = O * (1/s) as
    qt-batched DVE muls (the reciprocal column broadcasts along dh via a
    stride-0 AP), PE transpose (identity matmul) to X^T, row-parallel Wo
    partial, one bf16 out DMA per q-tile
Host: shards inputs (transposes q/kv once, casts bf16), sums the 4
bf16 partials per batch in f32, +bo.

Self-contained: hardcodes all shapes; requires concourse + numpy + ml_dtypes.
"""

import os

import numpy as np
import ml_dtypes

import concourse.bass as bass  # noqa: F401  (bass types via bacc/tile)
import concourse.tile as tile
from concourse import bacc, mybir
from concourse import bass_utils

F32 = mybir.dt.float32
F32R = mybir.dt.float32r
BF16 = mybir.dt.bfloat16
EXP = mybir.ActivationFunctionType.Exp
COPY = mybir.ActivationFunctionType.Copy

B, NQ, NKV, D = 2, 512, 4096, 1024
HEADS, DH = 16, 64
SCALE = DH ** -0.5
N_CORES = 8
HPC = HEADS // (N_CORES // B)   # heads per core = 4
PAIRS = HPC // 2                # head pairs per core = 2
KC = 512                        # Nkv chunk size
NCHUNK = NKV // KC              # 8
KO = D // 128                   # 8 contraction sub-chunks
NQT = NQ // 128                 # 4 q tiles
NPIECE = NCHUNK * 4             # 32 attention/projection pieces

KV_BUFS = int(os.environ.get("KV_BUFS", "2"))
KT_BUFS = int(os.environ.get("KT_BUFS", "3"))
V_BUFS = int(os.environ.get("V_BUFS", "8"))
AV_LAG = int(os.environ.get("AV_LAG", "14"))     # steady-state pending target
AV_HOARD = int(os.environ.get("AV_HOARD", "84"))  # pending before the drain
HOARD_AT = int(os.environ.get("HOARD_AT", "6"))  # piece where hoarding starts
DRAIN_AT = int(os.environ.get("DRAIN_AT", "24"))  # piece where draining starts
PT_BUFS = int(os.environ.get("PT_BUFS", "108"))
# PSUM layout: 4 pair-pure AV banks (pair0 -> banks 0/1, pair1 -> banks
# 2/3, 7+1 slots each) so neither pair's normalize waits on the other's
# accumulators (deps are tile-granular); QK (psS) gets the remaining 2.
PSS_BUFS = int(os.environ.get("PSS_BUFS", "2"))
C7_FILL = int(os.environ.get("C7_FILL", "4"))
OUT_BF16 = os.environ.get("OUT_BF16", "1") == "1"
_NC_CACHE = []


def _build_nc():
    nc = bacc.Bacc("TRN2", target_bir_lowering=False, debug=False,
                   num_devices=N_CORES)
    qT = nc.dram_tensor("qT", [D, NQ], BF16, kind="ExternalInput").ap()
    kvT = nc.dram_tensor("kvT", [D, NKV], BF16, kind="ExternalInput").ap()
    wq = nc.dram_tensor("wq", [D, HPC * DH], BF16, kind="ExternalInput").ap()
    wkvk = nc.dram_tensor("wkvk", [D, HPC * DH], BF16, kind="ExternalInput").ap()
    wkvv = nc.dram_tensor("wkvv", [D, HPC * DH], BF16, kind="ExternalInput").ap()
    wo = nc.dram_tensor("wo", [HPC * DH, D], BF16, kind="ExternalInput").ap()
    bias = nc.dram_tensor("bias", [128, NPIECE], F32, kind="ExternalInput").ap()
    ident = nc.dram_tensor("ident", [128, 128], BF16, kind="ExternalInput").ap()
    out_dt = BF16 if OUT_BF16 else F32
    out = nc.dram_tensor("out", [NQ, D], out_dt, kind="ExternalOutput").ap()

    qT_r = qT.rearrange("(ko p) n -> p ko n", p=128)
    kvT_r = kvT.rearrange("(ko p) n -> p ko n", p=128)
    wq_r = wq.rearrange("(ko p) m -> p ko m", p=128)
    wkvk_r = wkvk.rearrange("(ko p) m -> p ko m", p=128)
    wkvv_r = wkvv.rearrange("(ko p) m -> p ko m", p=128)
    wo_r = wo.rearrange("(ic p) n -> p ic n", p=128)

    with tile.TileContext(nc) as tc:
        with (
            tc.tile_pool(name="const", bufs=1) as cpool,
            tc.tile_pool(name="kv", bufs=KV_BUFS) as kv_pool,
            tc.tile_pool(name="kt", bufs=KT_BUFS) as kt_pool,
            tc.tile_pool(name="v", bufs=V_BUFS) as v_pool,
            tc.tile_pool(name="pt", bufs=PT_BUFS) as p_pool,
            tc.tile_pool(name="ob", bufs=4) as o_pool,
            tc.tile_pool(name="psA", bufs=1, space="PSUM") as psA,
            tc.tile_pool(name="psV", bufs=1, space="PSUM") as psV,
            tc.tile_pool(name="psS", bufs=PSS_BUFS, space="PSUM") as psS,
            tc.tile_pool(name="psO", bufs=1, space="PSUM") as psO,
        ):
            wq_sb = cpool.tile([128, KO, HPC * DH], BF16, tag="wq")
            wkvk_sb = cpool.tile([128, KO, HPC * DH], BF16, tag="wkvk")
            wkvv_sb = cpool.tile([128, KO, HPC * DH], BF16, tag="wkvv")
            wo_sb = cpool.tile([128, PAIRS, D], BF16, tag="wo")
            qT_sb = cpool.tile([128, KO, NQ], BF16, tag="qT")
            bias_sb = cpool.tile([128, NPIECE], F32, tag="bias")
            ident_sb = cpool.tile([128, 128], BF16, tag="ident")
            qh_sb = cpool.tile([128, PAIRS, NQ], F32R, tag="qh")
            xT_sb = cpool.tile([128, PAIRS, NQ], BF16, tag="xT")
            xn_all = cpool.tile([128, NQT, PAIRS, 2, DH], BF16, tag="xn")
            rt_sb = [cpool.tile([128, 7], F32, tag=f"rt{b}", name=f"rt{b}")
                     for b in range(4)]

            # prologue DMAs: K weights + chunk0 first (split fine so K-proj
            # starts ASAP and streams behind the DMA)
            kvc0 = kv_pool.tile([128, KO, KC], BF16, tag="kvc", name="kvc0")
            nc.sync.dma_start(wkvk_sb[:, 0:2, :], wkvk_r[:, 0:2, :])
            nc.sync.dma_start(kvc0[:, 0:2, :], kvT_r[:, 0:2, 0:KC])
            nc.sync.dma_start(wkvk_sb[:, 2:4, :], wkvk_r[:, 2:4, :])
            nc.sync.dma_start(kvc0[:, 2:4, :], kvT_r[:, 2:4, 0:KC])
            nc.sync.dma_start(wkvk_sb[:, 4:8, :], wkvk_r[:, 4:8, :])
            nc.sync.dma_start(kvc0[:, 4:6, :], kvT_r[:, 4:6, 0:KC])
            nc.sync.dma_start(wkvv_sb[:], wkvv_r)
            nc.sync.dma_start(kvc0[:, 6:8, :], kvT_r[:, 6:8, 0:KC])
            nc.sync.dma_start(wq_sb[:], wq_r)
            nc.sync.dma_start(qT_sb[:, 0:4, :], qT_r[:, 0:4, :])
            nc.sync.dma_start(qT_sb[:, 4:8, :], qT_r[:, 4:8, :])
            kvc1 = kv_pool.tile([128, KO, KC], BF16, tag="kvc", name="kvc1")
            nc.sync.dma_start(kvc1[:], kvT_r[:, :, KC:2 * KC])
            nc.sync.dma_start(bias_sb[:], bias)

            def q_projection():
                for p, pool in ((0, psA), (1, psV)):
                    qp = pool.tile([128, NQ], F32, tag=pool.name, name=f"qp{p}")
                    for ko in range(KO):
                        nc.tensor.matmul(
                            qp[:], wq_sb[:, ko, 128 * p:128 * (p + 1)],
                            qT_sb[:, ko, :], start=(ko == 0), stop=(ko == KO - 1),
                        )
                    nc.vector.tensor_copy(qh_sb[:, p, :], qp[:])

            # O accumulators: 16 groups (h, qt) of [128 q, DH+1] f32 in 4
            # pair-pure PSUM banks: pair0 (h0,h1) -> bank0 slots 0-6 + bank1
            # slot 0; pair1 (h2,h3) -> bank2/bank3 likewise. The first
            # matmul into each bank carries start=True; the pending-zero
            # region mechanism zeroes each group's first write.
            obank = [psO.tile([128, 512], F32, tag=f"ob{b}", name=f"obank{b}")
                     for b in range(4)]

            def o_place(h, qt):
                g = h * NQT + qt
                base, j = (2, g - 8) if g >= 8 else (0, g)
                if j == 7:
                    return base + 1, 0
                return base, j

            def o_slice(h, qt, w=DH + 1):
                bk, slot = o_place(h, qt)
                return obank[bk][:, 65 * slot:65 * slot + w], slot

            kvcs = {0: kvc0, 1: kvc1}

            def prefetch_kvc(c):
                if c in kvcs or c >= NCHUNK:
                    return
                kvc = kv_pool.tile([128, KO, KC], BF16, tag="kvc", name=f"kvc{c}")
                nc.sync.dma_start(kvc[:], kvT_r[:, :, KC * c:KC * (c + 1)])
                kvcs[c] = kvc

            def proj_k_pair(c, ktc, p, pool=None):
                kvc = kvcs[c]
                pool = pool or psA
                kp = pool.tile([128, KC], F32, tag=pool.name, name=f"kp{c}_{p}")
                for ko in range(KO):
                    nc.tensor.matmul(
                        kp[:], wkvk_sb[:, ko, 128 * p:128 * (p + 1)],
                        kvc[:, ko, :], start=(ko == 0), stop=(ko == KO - 1),
                    )
                nc.vector.tensor_copy(ktc[:, p, :], kp[:])

            def proj_v_sub(c, vc, m, pool=None):
                kvc = kvcs[c]
                pool = pool or psV
                vp = pool.tile([128, KC], F32, tag=pool.name, name=f"vp{c}_{m}")
                for ko in range(KO):
                    nc.tensor.matmul(
                        vp[:, 0:HPC * DH], kvc[:, ko, 128 * m:128 * (m + 1)],
                        wkvv_sb[:, ko, :], start=(ko == 0), stop=(ko == KO - 1),
                    )
                nc.vector.tensor_copy(
                    vc[:, m, :, 0:DH],
                    vp[:, 0:HPC * DH].rearrange("p (h d) -> p h d", h=HPC),
                )

            proj_tiles = {}

            def proj_k_both(c, ktc):
                # chunk-0 only: interleave both pairs ko-outer so each
                # arriving kv/weight strip feeds two matmuls, not one
                kvc = kvcs[c]
                kp0 = psA.tile([128, KC], F32, tag="psA", name=f"kpb{c}_0")
                kp1 = psS.tile([128, KC], F32, tag="psS", name=f"kpb{c}_1")
                for ko in range(KO):
                    for p, kp in ((0, kp0), (1, kp1)):
                        nc.tensor.matmul(
                            kp[:], wkvk_sb[:, ko, 128 * p:128 * (p + 1)],
                            kvc[:, ko, :], start=(ko == 0), stop=(ko == KO - 1),
                        )
                nc.vector.tensor_copy(ktc[:, 0, :], kp0[:])
                nc.vector.tensor_copy(ktc[:, 1, :], kp1[:])

            def proj_piece(gp, pools=(None, None)):
                if gp >= NPIECE:
                    return
                c, s = divmod(gp, 4)
                if s == 0:
                    ktc = kt_pool.tile([128, PAIRS, KC], F32R, tag="ktc", name=f"ktc{c}")
                    vc = v_pool.tile([128, 4, HPC, DH + 1], BF16, tag="vc", name=f"vc{c}")
                    nc.vector.memset(vc[:, :, :, DH:DH + 1], 1.0)
                    proj_tiles[c] = (ktc, vc)
                ktc, vc = proj_tiles[c]
                if s == 0:
                    proj_k_pair(c, ktc, 0, pool=pools[0])
                elif s == 1:
                    proj_k_pair(c, ktc, 1, pool=pools[0])
                elif s == 2:
                    proj_v_sub(c, vc, 0, pool=pools[0])
                    proj_v_sub(c, vc, 1, pool=pools[1])
                else:
                    proj_v_sub(c, vc, 2, pool=pools[0])
                    proj_v_sub(c, vc, 3, pool=pools[1])

            # AV entries split by head pair: pair1 (h2,h3) drains first so
            # its groups close before chunk 7's pair-0 attention, letting
            # half the normalize/transpose tail overlap the last QK window.
            pend01 = []
            pend23 = []

            def qk_exp_pair(c, s, p):
                ktc, vc = proj_tiles[c]
                bias_ap = bias_sb[:, 4 * c + s:4 * c + s + 1]
                sps = []
                for half in range(2):  # row-tiled pair, K=64
                    lo, hi = 64 * half, 64 * (half + 1)
                    sp = psS.tile([128, NQ], F32, tag="psS", name=f"sp{c}_{s}_{p}_{half}")
                    nc.tensor.matmul(
                        sp[:], ktc[lo:hi, p, 128 * s:128 * (s + 1)],
                        qh_sb[lo:hi, p, :], start=True, stop=True,
                    )
                    sps.append(sp)
                for half, sp in enumerate(sps):
                    h = 2 * p + half
                    pt = p_pool.tile([128, NQ], BF16, tag="pt", name=f"pt{c}_{s}_{p}_{half}")
                    nc.scalar.activation(
                        pt[:], sp[:], EXP, bias=bias_ap, scale=SCALE,
                    )
                    (pend23 if h >= 2 else pend01).append((c, s, h, vc, pt))

            def qk_exp_piece(c, s):
                for p in range(PAIRS):
                    qk_exp_pair(c, s, p)

            started_banks = set()

            def emit_av(entry):
                # the first matmul touching a bank carries start=True, which
                # marks the whole 2KB zero region pending; every other
                # group's first write then zeroes its own slot lazily
                c, s, h, vc, pt = entry
                for qt in range(NQT):
                    osl, slot = o_slice(h, qt)
                    bk, _ = o_place(h, qt)
                    nc.tensor.matmul(
                        osl, pt[:, 128 * qt:128 * (qt + 1)],
                        vc[:, s, h, :],
                        start=(bk not in started_banks),
                        stop=(c == NCHUNK - 1 and s == 3),
                        skip_group_check=True,
                    )
                    started_banks.add(bk)

            def flush_av(upto):
                # keep total pending <= upto, draining pair1 entries first
                while len(pend01) + len(pend23) > upto:
                    lst = pend23 if pend23 else pend01
                    emit_av(lst.pop(0))

            def flush_list(lst, upto=0):
                while len(lst) > upto:
                    emit_av(lst.pop(0))

            def av_target(a):
                if a < HOARD_AT:
                    return AV_LAG
                return AV_HOARD

            # prologue compute: chunk0 projections, Q projection, chunk1 K
            # prologue compute: chunk0 + chunk1-K projections and the Q
            # projection, psum spread across the not-yet-used psS banks so
            # consecutive pieces don't serialize on WAR copy dependencies
            proj_piece(0)
            proj_piece(1, pools=(psS, None))
            proj_piece(2, pools=(psV, psS))
            proj_piece(3, pools=(psS, psV))
            q_projection()
            proj_piece(4)
            proj_piece(5, pools=(psS, None))

            # steady state (chunks 0..6): attention piece a, projection a+6
            for a in range(NPIECE - 4):
                c, s = divmod(a, 4)
                if s == 0:
                    prefetch_kvc(c + 2)
                    if c == 2:
                        nc.sync.dma_start(wo_sb[:], wo_r)
                        nc.sync.dma_start(ident_sb[:], ident)
                qk_exp_piece(c, s)
                flush_av(av_target(a))
                proj_piece(a + 6)

            # chunk 7 endgame, pair-major: pair1 (h2,h3) attention first,
            # hoarded pair0 entries fill the PE while ACT runs the exps
            c7 = NCHUNK - 1
            for s in range(4):
                qk_exp_pair(c7, s, 1)
                proj_piece(NPIECE - 4 + s + 6)
                for _ in range(C7_FILL):
                    if pend01:
                        emit_av(pend01.pop(0))
            flush_list(pend23)

            tp_pools = [psA, psV]

            def norm_transpose(heads, engines):
                # normalize the given heads for all q-tiles in ONE
                # qt-batched mul per head (obank slots are qt-contiguous
                # per head in the pair-pure layout; the reciprocal column
                # broadcasts along dh via a stride-0 AP), then transpose
                # each q-tile's inner-pair block to X^T.
                ic = heads[0] // 2
                def bmul(h, qt0, nqt):
                    # one broadcast divide covering q-tiles qt0..qt0+nqt-1
                    # of head h (their obank slots are contiguous): the sums
                    # column broadcasts along dh via a stride-0 AP, skipping
                    # the reciprocal stage entirely
                    bk, slot0 = o_place(h, qt0)
                    blk = obank[bk][:, 65 * slot0:65 * (slot0 + nqt)] \
                        .rearrange("p (s w) -> p s w", w=65)
                    src_ap = blk[:, :, 0:DH]
                    sums3 = blk[:, :, 64:65]
                    src_b, sums_b = bass.broadcast_tensor_aps(src_ap, sums3)
                    nc.vector.tensor_tensor(
                        xn_all[:, qt0:qt0 + nqt, ic, h % 2, :], src_b, sums_b,
                        mybir.AluOpType.divide)

                for h in heads:
                    if h % 2 == 0:
                        bmul(h, 0, NQT)       # slots 0-3 of the 7-slot bank
                    else:
                        bmul(h, 0, NQT - 1)   # slots 4-6
                        bmul(h, NQT - 1, 1)   # overflow bank, slot 0
                for qt in range(NQT):
                    pool_t = tp_pools[qt % 2]
                    tp = pool_t.tile([128, 128], BF16, tag=pool_t.name,
                                     name=f"tp{qt}_{ic}")
                    nc.tensor.transpose(
                        tp[:], xn_all[:, qt, ic, :, :], ident_sb[:],
                    )
                    eng = engines[qt % len(engines)]
                    if eng == "dve":
                        nc.vector.tensor_copy(
                            xT_sb[:, ic, 128 * qt:128 * (qt + 1)], tp[:])
                    else:
                        nc.scalar.copy(
                            xT_sb[:, ic, 128 * qt:128 * (qt + 1)], tp[:])

            # pair1 tail: sums live in banks 2 (7 slots) + 3 (1 slot);
            # DVE-only so ACT stays free for pair0's exps
            sums2 = obank[2][:, 0:65 * 7].rearrange(
                "p (s w) -> p s w", w=65)[:, :, 64:65]
            nc.vector.reciprocal(rt_sb[2][:, 0:7], sums2)
            nc.vector.reciprocal(rt_sb[3][:, 0:1], obank[3][:, 64:65])
            norm_transpose((2, 3), ("dve",))

            # chunk 7 pair0 attention
            for s in range(4):
                qk_exp_pair(c7, s, 0)
                for _ in range(C7_FILL):
                    if pend01:
                        emit_av(pend01.pop(0))
            flush_list(pend01)

            # pair0 tail: bank0 (7 slots) + bank1 (1 slot)
            sums0 = obank[0][:, 0:65 * 7].rearrange(
                "p (s w) -> p s w", w=65)[:, :, 64:65]
            nc.vector.reciprocal(rt_sb[0][:, 0:7], sums0)
            nc.vector.reciprocal(rt_sb[1][:, 0:1], obank[1][:, 64:65])
            norm_transpose((0, 1), ("dve", "act"))
            wo_pools = [psS, psA, psV]
            for qt in range(NQT):
                osb = o_pool.tile([128, D], out_dt, tag="osb", name=f"osb{qt}")
                for n in range(D // 512):
                    j = qt * (D // 512) + n
                    pool_w = wo_pools[j % 3]
                    wp = pool_w.tile([128, 512], F32, tag=pool_w.name, name=f"wp{qt}_{n}")
                    for ic in range(PAIRS):
                        nc.tensor.matmul(
                            wp[:], xT_sb[:, ic, 128 * qt:128 * (qt + 1)],
                            wo_sb[:, ic, 512 * n:512 * (n + 1)],
                            start=(ic == 0), stop=(ic == PAIRS - 1),
                        )
                    if j % 2 == 1:
                        nc.scalar.copy(osb[:, 512 * n:512 * (n + 1)], wp[:])
                    else:
                        nc.vector.tensor_copy(osb[:, 512 * n:512 * (n + 1)], wp[:])
                nc.sync.dma_start(out[128 * qt:128 * (qt + 1), :], osb[:])

    nc.compile()
    return nc


def _get_nc():
    if not _NC_CACHE:
        _NC_CACHE.append(_build_nc())
    return _NC_CACHE[0]


LAST_RESULTS = None


def _bf16(x):
    return np.ascontiguousarray(x.astype(ml_dtypes.bfloat16))


def kernel(q, kv, mask, Wq, Wkv, Wo, bo):
    global LAST_RESULTS
    q = np.asarray(q, dtype=np.float32)
    kv = np.asarray(kv, dtype=np.float32)
    mask = np.asarray(mask)
    Wq = np.asarray(Wq, dtype=np.float32)
    Wkv = np.asarray(Wkv, dtype=np.float32)
    Wo = np.asarray(Wo, dtype=np.float32)
    bo = np.asarray(bo, dtype=np.float32)

    inner = HEADS * DH
    qT = [_bf16(q[b].T) for b in range(B)]
    kvT = [_bf16(kv[b].T) for b in range(B)]
    bias = []
    for b in range(B):
        bb = np.where(mask[b], 0.0, -30000.0).astype(np.float32)
        bias.append(np.ascontiguousarray(bb.reshape(NPIECE, 128).T))
    ident = np.eye(128, dtype=ml_dtypes.bfloat16)

    in_maps = []
    for i in range(N_CORES):
        b, g = divmod(i, N_CORES // B)
        cs = slice(HPC * DH * g, HPC * DH * (g + 1))
        in_maps.append({
            "qT": qT[b],
            "kvT": kvT[b],
            "wq": _bf16(Wq[:, cs]),
            "wkvk": _bf16(Wkv[:, cs]),
            "wkvv": _bf16(Wkv[:, inner:][:, cs]),
            "wo": _bf16(Wo[cs, :]),
            "bias": bias[b],
            "ident": ident,
        })

    nc = _get_nc()
    res = None
    for attempt in range(3):
        try:
            res = bass_utils.run_bass_kernel_spmd(
                nc, in_maps, core_ids=list(range(N_CORES)))
            break
        except Exception:
            if attempt == 2:
                raise
    LAST_RESULTS = res

    gpb = N_CORES // B
    out = np.zeros((B, NQ, D), np.float32)
    for b in range(B):
        acc = res.results[b * gpb]["out"].astype(np.float32).copy()
        for g in range(1, gpb):
            acc += res.results[b * gpb + g]["out"].astype(np.float32)
        out[b] = acc + bo[None, :]
    return out
